# revision 22
# baseline (speedup 1.0000x reference)
"""Trainium2 Bass kernel for a 3-layer stacked LSTM (nn_BlockLSTM).

Problem shapes (hardcoded): B=512, S=512, IN=H=128, 3 layers, fp32 I/O.
Sharding: data-parallel over batch across 8 cores (64 batch rows per core);
weights replicated; sequence stays local (recurrence).

Structure: 3-layer wavefront software pipeline — at wavefront t, layer l
processes step s = t - 2*l.  Each layer keeps its own small-instruction
chain; the steady-state period is bound by the per-step recurrence cycle
  mm(Whh@h) -> sigmoid(gates) -> [t2,t1,add] -> tanh(c) -> hmul -> mm
so every op on that cycle runs on the lowest-latency engine for it:
sigmoid/tanh on Act (table funcs), the elementwise triplet + hmul on DVE
(2x fp16 modes, small fixed access latency), matmuls on PE.

Per-core layout: feature-major: tiles are (128 partitions = feature,
64 free = batch). Gate order is [i, f, g, o] with the g block pre-scaled 2x
on the host, so ONE sigmoid activation covers all four gates (tanh(g) =
2*sig(2g)-1).  Bias is broadcast into PSUM by a C=4 matmul (bias rows x 0/1
indicator) that opens each accumulation group.

The cell state is tracked as C = c/2 in fp16 (the 2x folds into the tanh
scale operand) and is double-buffered by wavefront parity to kill the
tanh(t-1) -> add(t) WAR edge.

After finalize, a sync-retuning pass redistributes semaphore waits: the
Bacc event-semaphore splitter leaves the critical data wait on a
standalone EventSemaphore (which blocks the sequencer, serializing decode
behind the wait); we re-attach the most-recent-producer (min-slack) wait
to the engine instruction itself so decode/dispatch pipeline ahead and
the wait parks in the engine wait queue, and push the stale WAR waits to
the EventSemaphore, which then completes instantly.

Steady state runs at ~2166 ns/wavefront against two hard floors: Act
engine capacity (3 sigmoids + 3 tanhs = 1908 ns/wavefront; each Act
instruction carries ~185 ns of fixed SBUF-access latency, and no other
engine has transcendentals) and the recurrence-cycle latency (~2082 ns:
mm 314 + sigmoid 617 + c-update 505 — including a 95 ns producer
write-ack-to-semaphore stall that is REQUIRED on HW (engines pipeline
internally; eliding same-engine RAW waits was measured to corrupt
results on device) — + tanh 457 + h-mul 189).  The pinned schedule
(asymmetric phases 0/590/1300, tanh slotted into the Act gap after the
next layer's sigmoid) forces the packed service order on each engine.
"""

import numpy as np

B = 512
S = 512
H = 128
IN = 128
NCORES = 8
BC = B // NCORES  # 64 batch rows per core
NL = 3
TC = 4    # x-chunk steps per DMA load (layer 0)
TY = 4    # y staging steps per DMA store

_cache = {}

# sync retuning mode: 'off' | 'redist' | 'redist+elide'
# 'redist' relocates waits between EventSemaphore helpers and the engine
# instruction (semantically neutral). 'redist+elide' additionally drops
# same-engine counter waits — UNSAFE on HW (engines pipeline internally;
# back-to-back RAW through SBUF races), kept only for experiments.
SYNC_MODE = 'redist'


def _retune_sync(nc):
    """Redistribute waits between EventSemaphore helpers and their engine
    instruction: attach the min-slack (most recently produced) wait to the
    instruction, park stale WAR waits on the EventSemaphore."""
    from concourse import mybir

    if SYNC_MODE == 'off':
        return nc
    elide = SYNC_MODE == 'redist+elide'

    # engine -> its counter-sem name prefix (waits on your OWN engine's
    # counter are implied by in-order execution; dropping them lets a
    # consumer start at the producer's exec-end instead of waiting out the
    # producer's write-ack + semaphore propagation)
    eng_prefix = {
        mybir.EngineType.DVE: 'DVE_',
        mybir.EngineType.PE: 'PE_',
        mybir.EngineType.Activation: 'Activation_',
        mybir.EngineType.Pool: 'Pool_',
    }

    for fn in nc.m.functions:
        for blk in fn.blocks:
            insts = list(blk.instructions)
            # totals to estimate per-sem update rate (for slack normalization)
            def is_inc(u):
                m = str(u.update_mode)
                return 'inc' in m or 'add' in m

            total_upd = {}
            for i in insts:
                si = i.sync_info
                if si:
                    for u in si.on_update:
                        nm = u.ant_name
                        if nm and u.update_value and is_inc(u):
                            total_upd[nm] = total_upd.get(nm, 0) + u.update_value
            if not total_upd:
                continue
            nwf = max(1, getattr(nc, '_retune_nwf', 1))
            rate = {k: max(v / nwf, 1e-6) for k, v in total_upd.items()}

            upd_count = {}
            pending = {}  # engine -> [EventSemaphore...]
            for i in insts:
                eng = i.engine
                si = i.sync_info
                opc = i.opcode
                if opc == 'EventSemaphore':
                    pending.setdefault(eng, []).append(i)
                elif opc in ('DMACopy', 'Call', 'UnconditionalBranch', 'Drain',
                             'ISA', 'LoadActFuncSet'):
                    # don't touch DMA/control sync; their ES helpers stay put
                    pending.pop(eng, None)
                else:
                    es = pending.pop(eng, [])
                    pref = eng_prefix.get(eng) if elide else None
                    if es or (pref and si and any(
                            (w.ant_name or '').startswith(pref)
                            for w in si.on_wait)):
                        waits = []
                        for e in es:
                            esi = e.sync_info
                            if esi:
                                waits += list(esi.on_wait)
                        own_upd = []
                        if si:
                            waits += list(si.on_wait)
                            own_upd = list(si.on_update)
                        # drop same-engine counter waits (in-order implied)
                        if pref:
                            waits = [w for w in waits
                                     if not (w.ant_name or '').startswith(pref)]
                        if not waits:
                            i.sync_info = mybir.SyncInfo(
                                on_wait=[], on_update=own_upd)
                            for e in es:
                                esi = e.sync_info
                                e.sync_info = mybir.SyncInfo(
                                    on_wait=[],
                                    on_update=list(esi.on_update) if esi else [])
                        if waits:
                            def slack(w):
                                nm = w.ant_name or ''
                                if ('DMA' in nm or nm not in total_upd
                                        or w.wait_value is None):
                                    return 1e9
                                return ((upd_count.get(nm, 0) - w.wait_value)
                                        / rate.get(nm, 1.0))
                            waits.sort(key=slack)
                            attach, rest = waits[0], waits[1:]
                            i.sync_info = mybir.SyncInfo(
                                on_wait=[attach], on_update=own_upd)
                            for e in es:
                                esi = e.sync_info
                                take, rest = rest[:2], rest[2:]
                                e.sync_info = mybir.SyncInfo(
                                    on_wait=take,
                                    on_update=list(esi.on_update) if esi else [])
                            assert not rest, "more waits than ES capacity"
                # tally updates (program-order producer counts)
                si2 = i.sync_info
                if si2:
                    for u in si2.on_update:
                        nm = u.ant_name
                        if nm and u.update_value and is_inc(u):
                            upd_count[nm] = upd_count.get(nm, 0) + u.update_value
    return nc


def _build(s_steps, tune=None):
    import concourse.bass as bass
    import concourse.bacc as bacc
    import concourse.tile as tile
    from concourse import mybir

    f32 = mybir.dt.float32
    bf16 = mybir.dt.bfloat16
    fp16 = mybir.dt.float16
    cdt = fp16
    AF = mybir.ActivationFunctionType
    ALU = mybir.AluOpType

    nc = bacc.Bacc("TRN2", target_bir_lowering=False, debug=False)

    x_d = nc.declare_dram_parameter("x", [s_steps, IN, BC], bf16, isOutput=False)
    wih_d = [nc.declare_dram_parameter(f"wih{l}", [128, 512], bf16, isOutput=False)
             for l in range(NL)]
    whh_d = [nc.declare_dram_parameter(f"whh{l}", [128, 512], bf16, isOutput=False)
             for l in range(NL)]
    bmat_d = nc.declare_dram_parameter("bmat", [12, 128], bf16, isOutput=False)
    ind_d = nc.declare_dram_parameter("ind", [4, 256], bf16, isOutput=False)
    y_d = nc.declare_dram_parameter("y", [s_steps, H, BC], bf16, isOutput=True)

    with tile.TileContext(nc) as tc:
        with (
            tc.tile_pool(name="wpool", bufs=1) as wpool,
            tc.tile_pool(name="xst", bufs=3) as xpool,
            tc.tile_pool(name="yst", bufs=2) as ypool,
            tc.tile_pool(name="state", bufs=1) as spool,
            tc.tile_pool(name="psum", bufs=2, space="PSUM") as pspool,
            tc.tile_pool(name="sig", bufs=8) as sigpool,
            tc.tile_pool(name="tmp1", bufs=8) as t1pool,
            tc.tile_pool(name="tmp2", bufs=8) as t2pool,
            tc.tile_pool(name="tc_", bufs=8) as tcpool,
        ):
            # --- resident weights (loaded once) ---
            wih_t = [wpool.tile([128, 512], bf16, name=f"wih{l}", tag=f"wih{l}")
                     for l in range(NL)]
            whh_t = [wpool.tile([128, 512], bf16, name=f"whh{l}", tag=f"whh{l}")
                     for l in range(NL)]
            nc.sync.dma_start(wih_t[0][:], wih_d[0][:])
            nc.sync.dma_start(whh_t[0][:], whh_d[0][:])
            bmat_t = wpool.tile([4, NL * 128], bf16, tag="bmat")
            nc.sync.dma_start(
                bmat_t[:], bass.AP(bmat_d, 0, [[128, 4], [512, NL], [1, 128]])
            )
            ind_t = wpool.tile([4, 256], bf16, tag="ind")
            nc.sync.dma_start(ind_t[:], ind_d[:])

            h_all = [spool.tile([128, NL * BC], bf16, name=f"h{i}", tag=f"h{i}")
                     for i in range(3)]
            for i in range(3):
                nc.vector.memset(h_all[i][:], 0.0)
            cbufs = [spool.tile([128, NL * BC], cdt, name=f"c{i}", tag=f"c{i}")
                     for i in range(2)]
            for i in range(2):
                nc.vector.memset(cbufs[i][:], 0.0)

            # Virtual-time phase pinning: force the static per-engine
            # instruction order to the packed steady-state schedule
            # (layers staggered P/3; each tanh slotted into the Act gap
            # after the next layer's sigmoid).  Pins shape static order
            # only; runtime sync stays semaphore-driven.
            _tune = {
                "P_RT": 2078,
                "SCALE": 2.0,      # 0 disables pins (greedy scheduler)
                "SP1": 590,        # asymmetric layer phase offsets (searched)
                "SP2": 1300,
                "O_MMPRE": -2200,
                "O_MMWHH": -312,
                "O_SIG": 0,
                "O_T2": 616,
                "O_T1": 710,
                "O_ADD": 838,
                "O_TANH": 1091,
                "O_HMUL": 1547,
                "O_YCP": 1640,
            }
            if tune:
                _tune.update(tune)
            P_RT = _tune["P_RT"]
            SCALE = _tune["SCALE"]
            SP = P_RT // 3
            SP1 = _tune.get("SP1", SP)
            SP2 = _tune.get("SP2", 2 * SP)
            YC_POOL = _tune.get("YC_POOL", False)
            T2_POOL = _tune.get("T2_POOL", False)
            WARM_T = _tune.get("WARM_T", -1)
            WARM = _tune.get("WARM", (0, 0, 0))

            def pin(rt_ns):
                return tc.tile_wait_until(max(rt_ns * SCALE, 0.0) * 1e-6)

            # dummy tiles for warmup phase-shaping kicks (Act copies)
            dummy = spool.tile([128, 512], bf16, tag="dummy")

            # x chunk tiles, prefetched one chunk ahead
            xtiles = {}

            def load_chunk(k):
                t0 = k * TC
                if t0 >= s_steps:
                    return
                nst = min(TC, s_steps - t0)
                xt = xpool.tile([128, TC * BC], bf16, tag="xst")
                nc.sync.dma_start(
                    xt[:, : nst * BC],
                    bass.AP(x_d, t0 * IN * BC,
                            [[BC, 128], [IN * BC, nst], [1, BC]]),
                )
                xtiles[k] = xt

            load_chunk(0)
            load_chunk(1)

            yst = None
            D = 2  # layer offset: layer l processes step s = t - D*l
            n_wf = s_steps + D * (NL - 1)
            for t in range(n_wf):
                lo = max(0, -(-(t - (s_steps - 1)) // D))
                hi = min(NL - 1, t // D)
                hrec = h_all[(t + 2) % 3]   # written at wavefront t-1
                hin = h_all[(t + 1) % 3]    # written at wavefront t-2
                hcur = h_all[t % 3]
                cw = cbufs[t % 2]           # c written this wavefront
                cr = cbufs[(t + 1) % 2]     # c from wavefront t-1

                # ---- layer-0 input prefetch: one chunk ahead ----
                if t < s_steps and t % TC == 0 and t >= TC:
                    with pin((t - 6) * P_RT):
                        load_chunk(t // TC + 1)

                if t == 1:
                    for l in range(1, NL):
                        nc.sync.dma_start(wih_t[l][:], wih_d[l][:])
                        nc.sync.dma_start(whh_t[l][:], whh_d[l][:])

                spl = {0: 0, 1: SP1, 2: SP2}
                # warmup ramp: pace the pipeline-fill wavefronts at a
                # compressed virtual period so the scheduler packs them
                # densely; rejoin the steady grid at t == WRAMP_N
                WRAMP_N = _tune.get("WRAMP_N", 0)
                WRAMP_P = _tune.get("WRAMP_P", P_RT)
                if t < WRAMP_N:
                    tbase = WRAMP_N * P_RT - (WRAMP_N - t) * WRAMP_P
                else:
                    tbase = t * P_RT
                phis = {l: tbase + spl[l] for l in range(lo, hi + 1)}
                sigs, t1s, t2s, tcs, pss = {}, {}, {}, {}, {}
                for l in range(lo, hi + 1):
                    s = t - D * l
                    if l == 0:
                        xst = xtiles[s // TC]
                        x_ap = xst[:, (s % TC) * BC:(s % TC + 1) * BC]
                    else:
                        x_ap = hin[:, (l - 1) * BC:l * BC]
                    h_ap = hrec[:, l * BC:(l + 1) * BC]

                    # bias + Wih mms prefetch; only Whh mms sit on the h-loop
                    ps = pspool.tile([128, 256], f32, tag=f"ps{l}", name=f"ps{l}",
                                     bufs=3 if l < 2 else 2)
                    pss[l] = ps
                    with pin(phis[l] + _tune["O_MMPRE"]):
                        nc.tensor.matmul(
                            ps[:], bmat_t[:, l * 128:(l + 1) * 128], ind_t[:],
                            start=True, stop=False, skip_group_check=True,
                        )
                        for g in range(4):
                            nc.tensor.matmul(
                                ps[:, g * BC:(g + 1) * BC],
                                wih_t[l][:, g * 128:(g + 1) * 128], x_ap,
                                start=False, stop=False, skip_group_check=True,
                            )
                    with pin(phis[l] + _tune["O_MMWHH"]):
                        for g in range(4):
                            nc.tensor.matmul(
                                ps[:, g * BC:(g + 1) * BC],
                                whh_t[l][:, g * 128:(g + 1) * 128], h_ap,
                                start=False, stop=(g == 3), skip_group_check=True,
                            )

                for l in range(lo, hi + 1):
                    if t == WARM_T and WARM[l] > 0:
                        # one-time Act-stream delay: shifts layer l's phase
                        with pin(phis[l] + _tune["O_SIG"] - 2):
                            nc.scalar.activation(
                                dummy[:, :WARM[l]], dummy[:, :WARM[l]],
                                AF.Copy)
                    ps = pss[l]
                    sig = sigpool.tile([128, 256], fp16, tag=f"sig{l}",
                                       name=f"sig{l}")
                    with pin(phis[l] + _tune["O_SIG"]):
                        nc.scalar.activation(sig[:], ps[:], AF.Sigmoid)
                    sigs[l] = sig

                for l in range(lo, hi + 1):
                    t2 = t2pool.tile([128, BC], cdt, tag=f"t2{l}", name=f"t2{l}")
                    t2eng = nc.gpsimd if T2_POOL else nc.vector
                    with pin(phis[l] + _tune["O_T2"]):
                        t2eng.tensor_mul(
                            t2[:], sigs[l][:, BC:2 * BC],
                            cr[:, l * BC:(l + 1) * BC])
                    t2s[l] = t2
                    # t1 = (sig(2g) - 0.5) * sig(i) = sig(i)*tanh(g)/2
                    t1 = t1pool.tile([128, BC], fp16, tag=f"t1{l}", name=f"t1{l}")
                    with pin(phis[l] + _tune["O_T1"]):
                        nc.vector.scalar_tensor_tensor(
                            t1[:], sigs[l][:, 2 * BC:3 * BC], 0.5,
                            sigs[l][:, 0:BC], ALU.subtract, ALU.mult)
                    t1s[l] = t1
                for l in range(lo, hi + 1):
                    with pin(phis[l] + _tune["O_ADD"]):
                        nc.vector.tensor_add(
                            cw[:, l * BC:(l + 1) * BC], t1s[l][:], t2s[l][:])
                for l in range(lo, hi + 1):
                    tc_t = tcpool.tile([128, BC], bf16, tag=f"tc{l}",
                                       name=f"tc{l}")
                    with pin(phis[l] + _tune["O_TANH"]):
                        nc.scalar.activation(
                            tc_t[:], cw[:, l * BC:(l + 1) * BC], AF.Tanh,
                            scale=2.0)
                    tcs[l] = tc_t
                # h = sig(o) * tanh(c) on DVE (lowest-latency engine)
                for l in range(lo, hi + 1):
                    with pin(phis[l] + _tune["O_HMUL"]):
                        nc.vector.tensor_mul(
                            hcur[:, l * BC:(l + 1) * BC],
                            sigs[l][:, 3 * BC:4 * BC], tcs[l][:])

                # ---- output: layer 2's h -> bf16 staging -> DRAM ----
                if t >= D * (NL - 1):
                    s2 = t - D * (NL - 1)
                    if s2 % TY == 0:
                        yst = ypool.tile([128, TY * BC], bf16, tag="yst")
                    yeng = nc.gpsimd if YC_POOL else nc.vector
                    with pin(phis[NL - 1] + _tune["O_YCP"]):
                        yeng.tensor_copy(
                            yst[:, (s2 % TY) * BC:(s2 % TY + 1) * BC],
                            hcur[:, (NL - 1) * BC:NL * BC])
                    if s2 % TY == TY - 1 or s2 == s_steps - 1:
                        t0 = (s2 // TY) * TY
                        nst = s2 - t0 + 1
                        with pin(phis[NL - 1] + _tune["O_YCP"] + 150):
                            nc.sync.dma_start(
                                bass.AP(y_d, t0 * H * BC,
                                        [[BC, 128], [H * BC, nst], [1, BC]]),
                                yst[:, : nst * BC],
                            )
    nc.finalize()
    nc._retune_nwf = n_wf
    _retune_sync(nc)
    return nc


def _get_nc(s_steps):
    if s_steps not in _cache:
        _cache[s_steps] = _build(s_steps)
    return _cache[s_steps]


# gate reorder: pytorch [i, f, g, o] -> kernel [i, f, o, g]
_PERM = [0, 1, 2, 3]


def _prep_weights(Wih, Whh, bih, bhh):
    """Returns (wihT, whhT, brows) with gate blocks reordered to [i,f,o,g]
    and the g block scaled by 2 (tanh(g) = 2*sigmoid(2g) - 1 trick).

    wihT/whhT: (128, 512) f32 — W.T with columns grouped per gate.
    brows: (4, 128) f32 — bias row per (reordered) gate.
    """
    WihT = Wih.astype(np.float32).T  # (in, 4H)
    WhhT = Whh.astype(np.float32).T
    b = (bih + bhh).astype(np.float32)
    wcols_i, wcols_h, brows = [], [], []
    for k, g in enumerate(_PERM):
        scale = 2.0 if k == 2 else 1.0
        wcols_i.append(scale * WihT[:, g * H:(g + 1) * H])
        wcols_h.append(scale * WhhT[:, g * H:(g + 1) * H])
        brows.append(scale * b[g * H:(g + 1) * H])
    return (np.concatenate(wcols_i, axis=1), np.concatenate(wcols_h, axis=1),
            np.stack(brows))


def prepare_in_maps(inputs):
    import ml_dtypes

    bf = ml_dtypes.bfloat16
    x = np.asarray(inputs["x"], dtype=np.float32)  # (B, S, IN)
    s_steps = x.shape[1]

    wihTs, whhTs, bmats = [], [], []
    for l in range(3):
        wihT, whhT, brows = _prep_weights(
            np.asarray(inputs[f"Wih{l}"]), np.asarray(inputs[f"Whh{l}"]),
            np.asarray(inputs[f"bih{l}"]), np.asarray(inputs[f"bhh{l}"]))
        wihTs.append(wihT.astype(bf))
        whhTs.append(whhT.astype(bf))
        bmats.append(brows)
    bmat = np.concatenate(bmats, axis=0).astype(bf)  # (12, 128)
    ind = np.zeros((4, 256), dtype=np.float32)
    for g in range(4):
        ind[g, g * BC:(g + 1) * BC] = 1.0
    ind = ind.astype(bf)

    in_maps = []
    for c in range(NCORES):
        xc = x[c * BC:(c + 1) * BC]          # (BC, S, IN)
        xc = np.ascontiguousarray(xc.transpose(1, 2, 0)).astype(bf)  # (S, IN, BC)
        m = {"x": xc, "bmat": bmat, "ind": ind}
        for l in range(3):
            m[f"wih{l}"] = wihTs[l]
            m[f"whh{l}"] = whhTs[l]
        in_maps.append(m)
    return in_maps, s_steps


def kernel(**inputs):
    from concourse.bass_utils import run_bass_kernel_spmd

    in_maps, s_steps = prepare_in_maps(inputs)
    nc = _get_nc(s_steps)
    res = run_bass_kernel_spmd(nc, in_maps, list(range(NCORES)))

    y = np.empty((s_steps, H, B), dtype=np.float32)
    for c in range(NCORES):
        y[:, :, c * BC:(c + 1) * BC] = np.asarray(
            res.results[c]["y"]).astype(np.float32)
    return y
